# revision 29
# baseline (speedup 1.0000x reference)
"""Trainium2 Bass kernel for nn_LutLayer (6-bit Bernoulli-mixture LUT layer).

Math: with u_j = x_j + eps, v_j = (1 - x_j) + eps,
  lut_p[b,d,i] = prod_j (v_j if bit_j(i) else u_j)      (bit_j = MSB-first)
  out[b,d]     = sum_i sigmoid(50*lut[d,i]) * lut_p[b,d,i]

Split i = (h, l) with h = i >> 3 (bits of j=0,1,2), l = i & 7 (j=3,4,5):
  lut_p[i] = A_h * B_l,  A/B = exp of 3-term log sums
  out[b,d] = sum_h A_h * (sum_l G[d,h,l] * B_l),  G[d,h,l] = gate[d, 8h+l]

Device pipeline per (16-depth block, batch chunk):
  LU = Ln(x + eps), LV = Ln(-x + (1+eps))              [Scalar engine]
  SLB = PATBU.T@LU + PATBV.T@LV  (log-sum, 0/1 consts) [Tensor engine]
  SLA = PATAU.T@LU + PATAV.T@LV
  B = Exp(SLB), A = Exp(SLA)                           [Scalar engine]
  C = Wk.T @ B   (Wk = blockdiag sigmoid(50*lut))      [Tensor engine]
  P = A * C                                            [Vector engine]
  out = RPAT.T @ P  (sum over h per depth row)         [Tensor engine]

Sharding: depth-parallel across 8 cores (256 depth rows each, full batch).
Host does layout-only transforms (transpose/interleave/blockdiag scatter).
"""

import os
import sys

import numpy as np

for _p in ("/opt/trn_rl_repo", os.path.expanduser("~/.axon_site/_ro/trn_rl_repo")):
    if os.path.isdir(_p) and _p not in sys.path:
        sys.path.insert(0, _p)

import concourse.mybir as mybir  # noqa: E402
from concourse import bacc  # noqa: E402
from concourse.tile import TileContext  # noqa: E402

F32 = mybir.dt.float32
F32R = mybir.dt.float32r
F16 = mybir.dt.float16
AFT = mybir.ActivationFunctionType

# ---------------------------------------------------------------------------
# Activation-table pinning: by default the table-load pass picks a different
# act-func table for Ln vs Exp, so alternating Ln/Exp reloads the table every
# unit (~1.3us each, dominates the kernel). Strip Ln/Exp/Sigmoid from every
# table except one that serves each, so both Ln and Exp resolve to the shared
# "natural_log_exp_and_others" table (list order, and thus act_func_set_id,
# is preserved).
_GAT_PATCHED = False


def _patch_activation_tables():
    global _GAT_PATCHED
    if _GAT_PATCHED:
        return
    _GAT_PATCHED = True
    orig = bacc.get_activation_tables

    def patched(arch):
        tabs = orig(arch)
        keep = {"natural_log_exp_and_others", "sigmoid_and_others"}
        strip = {AFT.Ln, AFT.Exp, AFT.Sigmoid}
        return {
            name: (funcs if name in keep else (set(funcs) - strip))
            for name, funcs in tabs.items()
        }

    bacc.get_activation_tables = patched

SIX = 6
LUT_SCALE = 50.0
EPS = 1e-7
NEG_FILL = -30000.0  # *50 under sigmoid -> exactly 0; fits fp16
N_CORES = 8


def _bit(val: int, pos_msb_first: int, width: int = 3) -> int:
    """bit of `val` indexed MSB-first within `width` bits."""
    return (val >> (width - 1 - pos_msb_first)) & 1


def build_patterns(dl_blk: int = 16):
    """Constant 0/1 matmul patterns for the merged u/v log-sum stage.

    K layout: p = dl*6 + jj*2 + uv (96 rows; x staged duplicated so uv=0
    rows hold log(x+eps) and uv=1 rows log(1-x+eps)). M: (dl, code) =
    dl*8 + code. v is used when the code bit is 1 (p_q = [1-x, x] concat).
    """
    k = dl_blk * SIX
    patb = np.zeros((k, dl_blk * 8), np.float16)
    pata = np.zeros((k, dl_blk * 8), np.float16)
    for dl in range(dl_blk):
        for code in range(8):
            for jj in range(3):
                bit = _bit(code, jj)
                c = dl * 8 + code
                patb[dl * SIX + jj * 2 + bit, c] = 1.0
                pata[dl * SIX + jj * 2 + bit, c] = 1.0
    return patb, pata


def build_lnvecs(dl_blk: int = 16):
    """Per-partition scale/bias for the single Ln pass over duplicated x."""
    scale = np.zeros((96, 1), np.float32)
    bias = np.zeros((96, 1), np.float32)
    for p in range(96):
        if p % 2 == 0:
            scale[p] = 1.0
            bias[p] = EPS
        else:
            scale[p] = -1.0
            bias[p] = 1.0 + EPS
    return scale, bias


def build_rpat(g_sz: int, dl_blk: int = 16):
    """rpat8[g, (dl,h), (kk,dl')] = 1 iff kk==g and dl==dl' (h summed out).

    Used as lhsT of accumulating matmuls so g_sz k-blocks' outputs land in
    disjoint 16-partition strips of one PSUM tile.
    """
    rp = np.zeros((g_sz, dl_blk * 8, g_sz * dl_blk), np.float16)
    for g in range(g_sz):
        for dl in range(dl_blk):
            rp[g, dl * 8 : dl * 8 + 8, g * dl_blk + dl] = 1.0
    return rp


def host_prep(inputs: np.ndarray, lut: np.ndarray, d0: int, dc: int):
    """Layout-only transforms for one core owning depth rows [d0, d0+dc)."""
    b = inputs.shape[0]
    kb = dc // 16
    # xtb/xta[k, dl*6 + jj*2 + uv, b] = inputs[b, d0+16k+dl, jbase+jj] for
    # both uv slots (duplicated so one Ln pass computes log u and log v).
    xs = inputs[:, d0 : d0 + dc, :]  # (B, dc, 6)
    x4 = (
        xs.reshape(b, kb, 16, SIX).transpose(1, 2, 3, 0).astype(np.float16)
    )  # [k, dl, j, b]
    dup = np.repeat(x4, 2, axis=2)  # [k, dl, j*2(uv), b]
    xta = np.ascontiguousarray(dup[:, :, 0:6].reshape(kb, 96, b))
    xtb = np.ascontiguousarray(dup[:, :, 6:12].reshape(kb, 96, b))
    # lutbd[k, dl*8+l, dl*8+h] = lut[d, 8h+l], off-diagonal filled with NEG_FILL
    lt = lut[d0 : d0 + dc].reshape(kb, 16, 8, 8)  # [k, dl, h, l]
    lutbd = np.full((kb, 128, 128), NEG_FILL, np.float16)
    for dl in range(16):
        lutbd[:, dl * 8 : dl * 8 + 8, dl * 8 : dl * 8 + 8] = lt[:, dl].transpose(
            0, 2, 1
        )
    return xtb, xta, np.ascontiguousarray(lutbd)


def build_nc(dc: int, b: int, n_chunk: int):
    """Build the Bass program for one core: dc depth rows, b batch, chunks of n_chunk."""
    kb = dc // 16
    nb = b // n_chunk
    _patch_activation_tables()
    nc = bacc.Bacc("TRN2", target_bir_lowering=False, debug=False)

    def mm(out, lhsT, rhs, start, stop):
        # fp16 operands: PE runs 1 cycle/row (fp32 is 4) and the clock-warmup
        # monitor engages; log-sum rounding to fp16 costs ~0.1% output error.
        nc.tensor.matmul(out, lhsT, rhs, start=start, stop=stop)
    # Register activation-bias constants (only 0.0/1.0 exist by default).
    for val in (EPS, 1.0 + EPS):
        t = nc.alloc_sbuf_tensor(f"const-float32-{val}", [128, 1], F32)
        nc.gpsimd.memset(t.ap(), val)
        nc.const_aps.aps[(F32, val)] = t.ap()
    nc.all_engine_barrier()
    xtb_t = nc.declare_dram_parameter("xtb", [kb, 96, b], F16, isOutput=False)
    xta_t = nc.declare_dram_parameter("xta", [kb, 96, b], F16, isOutput=False)
    lutbd_t = nc.declare_dram_parameter("lutbd", [kb, 128, 128], F16, isOutput=False)
    patb_t = nc.declare_dram_parameter("patb", [96, 128], F16, isOutput=False)
    pata_t = nc.declare_dram_parameter("pata", [96, 128], F16, isOutput=False)
    lnscale_t = nc.declare_dram_parameter("lnscale", [96, 1], F32, isOutput=False)
    lnbias_t = nc.declare_dram_parameter("lnbias", [96, 1], F32, isOutput=False)
    g_sz = min(8, kb)
    rpat_t = nc.declare_dram_parameter(
        "rpat8", [g_sz, 128, g_sz * 16], F16, isOutput=False
    )
    out_t = nc.declare_dram_parameter("outT", [dc, b], F32, isOutput=True)

    with TileContext(nc) as tc:
        with (
            tc.tile_pool(name="const", bufs=1) as cpool,
            tc.tile_pool(name="io", bufs=4) as io,
            tc.tile_pool(name="act", bufs=4) as actp,
            tc.tile_pool(name="ps", bufs=2, space="PSUM") as ps,
            tc.tile_pool(name="psc", bufs=2, space="PSUM") as psc,
            tc.tile_pool(name="pso", bufs=2, space="PSUM") as pso,
        ):
            pats = {}
            for name, t in (("patb", patb_t), ("pata", pata_t)):
                s = cpool.tile([96, 128], F16, tag=name)
                nc.sync.dma_start(s, t[:, :])
                pats[name] = s
            lnscale = cpool.tile([96, 1], F32, tag="lnscale")
            nc.sync.dma_start(lnscale, lnscale_t[:, :])
            lnbias = cpool.tile([96, 1], F32, tag="lnbias")
            nc.sync.dma_start(lnbias, lnbias_t[:, :])
            rpats = []
            for g in range(g_sz):
                s = cpool.tile([128, g_sz * 16], F16, tag=f"rpat{g}")
                nc.sync.dma_start(s, rpat_t[g, :, :])
                rpats.append(s)

            # All gate weights in one tile: one DMA + one Sigmoid (keeps the
            # act-table switch count low for the whole kernel).
            wraw = io.tile([128, kb * 128], F16, tag="wraw")
            nc.sync.dma_start(
                wraw.rearrange("p (k m) -> p k m", k=kb),
                lutbd_t.ap().rearrange("k p m -> p k m"),
            )
            wkall = cpool.tile([128, kb * 128], F16, tag="wkall")
            nc.scalar.activation(wkall, wraw, AFT.Sigmoid, scale=LUT_SCALE)

            for grp in range(kb // g_sz):
                for n in range(nb):
                    sl = slice(n * n_chunk, (n + 1) * n_chunk)
                    # One strided DMA per side gathers this (grp, n) slice
                    # for all g_sz k-blocks; one Ln op per side covers both
                    # log(x+eps) and log(1-x+eps) via per-partition scale/bias
                    # over the uv-duplicated staging.
                    luvb = actp.tile([96, g_sz * n_chunk], F16, tag="luvb")
                    luva = actp.tile([96, g_sz * n_chunk], F16, tag="luva")
                    for xtsrc, dst in ((xtb_t, luvb), (xta_t, luva)):
                        xsg = io.tile([96, g_sz * n_chunk], F16, tag="xsg")
                        nc.sync.dma_start(
                            xsg.rearrange("p (k n) -> p k n", k=g_sz),
                            xtsrc[grp * g_sz : (grp + 1) * g_sz, :, sl].rearrange(
                                "k p n -> p k n"
                            ),
                        )
                        # (x*±1 + bias) on DVE (4x-mode fp16) so the Ln runs
                        # with immediate scale/bias (per-partition AP params
                        # cost ~700ns/op on the Scalar engine).
                        uvg = io.tile([96, g_sz * n_chunk], F16, tag="uvg")
                        nc.vector.tensor_scalar(
                            uvg,
                            xsg,
                            lnscale,
                            lnbias,
                            mybir.AluOpType.mult,
                            mybir.AluOpType.add,
                        )
                        nc.scalar.activation(dst, uvg, AFT.Ln)

                    ot = pso.tile([g_sz * 16, n_chunk], F32, tag="ot")

                    def emit_head(kk):
                        ks = slice(kk * n_chunk, (kk + 1) * n_chunk)
                        s = ps.tile([128, 2 * n_chunk], F32, tag="sl2")
                        mm(s[:, 0:n_chunk], pats["patb"], luvb[:, ks], True, True)
                        mm(
                            s[:, n_chunk : 2 * n_chunk],
                            pats["pata"],
                            luva[:, ks],
                            True,
                            True,
                        )
                        return s

                    def emit_tail(kk, s):
                        k = grp * g_sz + kk
                        ba2 = actp.tile([128, 2 * n_chunk], F16, tag="ba2")
                        nc.scalar.activation(ba2, s, AFT.Exp)
                        ct = psc.tile([128, n_chunk], F32, tag="ct")
                        mm(
                            ct,
                            wkall[:, k * 128 : (k + 1) * 128],
                            ba2[:, 0:n_chunk],
                            True,
                            True,
                        )
                        pt = io.tile([128, n_chunk], F16, tag="pt")
                        nc.vector.tensor_mul(
                            pt, ba2[:, n_chunk : 2 * n_chunk], ct
                        )
                        mm(ot, rpats[kk], pt, kk == 0, kk == g_sz - 1)

                    # one-unit software pipeline skew: the next unit's
                    # log-sum matmuls are issued before this unit's
                    # exp-dependent tail so the Scalar engine never starves.
                    pending = None
                    for kk in range(g_sz + 1):
                        nxt = (kk, emit_head(kk)) if kk < g_sz else None
                        if pending is not None:
                            emit_tail(*pending)
                        pending = nxt
                    stage = io.tile([g_sz * 16, n_chunk], F32, tag="stage")
                    nc.vector.tensor_copy(stage, ot)
                    nc.sync.dma_start(
                        out_t[grp * g_sz * 16 : (grp + 1) * g_sz * 16, sl], stage
                    )
    nc.finalize()
    return nc


def prepare(inputs: np.ndarray, lut: np.ndarray, p_q_2_lut_table: np.ndarray):
    """Build the Bass program and per-core input maps (host, layout only)."""
    inputs = np.ascontiguousarray(inputs, np.float32)
    lut = np.ascontiguousarray(lut, np.float32)
    b, d, six = inputs.shape
    assert six == SIX and d % (16 * N_CORES) == 0

    # Sanity: the table must be the canonical 6-bit indicator matrix this
    # kernel's constant patterns assume (it is, by construction).
    exp_table = np.zeros((2 * SIX, 2**SIX), np.float32)
    for i in range(2**SIX):
        for j in range(SIX):
            if (i >> (SIX - 1 - j)) & 1:
                exp_table[j, i] = 1.0
            else:
                exp_table[j + SIX, i] = 1.0
    assert np.array_equal(np.asarray(p_q_2_lut_table), exp_table), (
        "p_q_2_lut_table does not match the canonical bit-indicator layout"
    )

    dc = d // N_CORES
    n_chunk = 512 if b % 512 == 0 else b
    nc = build_nc(dc, b, n_chunk)

    patb, pata = build_patterns()
    lnscale, lnbias = build_lnvecs()
    rpat8 = build_rpat(min(8, dc // 16))
    in_maps = []
    for c in range(N_CORES):
        xtb, xta, lutbd = host_prep(inputs, lut, c * dc, dc)
        in_maps.append(
            {
                "xtb": xtb,
                "xta": xta,
                "lutbd": lutbd,
                "patb": patb,
                "pata": pata,
                "lnscale": lnscale,
                "lnbias": lnbias,
                "rpat8": rpat8,
            }
        )
    return nc, in_maps, (b, d, dc)


def gather(res_results, b, d, dc):
    out = np.empty((b, d), np.float32)
    for c in range(N_CORES):
        out[:, c * dc : (c + 1) * dc] = res_results[c]["outT"].T
    return out


def kernel(inputs: np.ndarray, lut: np.ndarray, p_q_2_lut_table: np.ndarray):
    nc, in_maps, (b, d, dc) = prepare(inputs, lut, p_q_2_lut_table)

    from concourse.bass_utils import run_bass_kernel_spmd

    res = run_bass_kernel_spmd(nc, in_maps, list(range(N_CORES)))
    return gather(res.results, b, d, dc)


if __name__ == "__main__":
    rng = np.random.default_rng(0)
    x = rng.random((256, 128, 6), dtype=np.float32)
    print("smoke test requires full-size inputs; use test.py")
